# revision 11
# baseline (speedup 1.0000x reference)
"""Trainium2 Bass kernel for nn_DynamicQuantizedLinear.

Computes out = x @ dequant(W).T + bias + residual where
  x:[64,4096] f32, W_q:[11008,4096] int8, scale:[11008,32] f16 (group size 128),
  bias/residual:[11008] f16.

Strategy (column-parallel over out_features, 8 cores):
  - Host: requantize the dequantized weights to a SINGLE int8 scale per
    output row: S[o] = max_g scale[o,g], w' = rint(dequant(W)/S) in [-127,127].
    This removes all per-group scaling from the device (rel err ~6e-3 vs
    2e-2 tolerance). Host applies out*S + bias + residual afterwards (free).
  - Device: weights stream as INT8 (5.6MB/core, half the fp16 bytes, ~14us
    at ~410GB/s) in 16 column chunks; DVE/ACT/GPSIMD round-robin cast each
    [128,1376] group slab int8->fp16 (~14.5us/engine, overlapped); PE
    accumulates all 32 K-groups into 3 PSUM banks (N=512/512/352) with a
    [128,64] fp16 x-tile stationary per group (~18.4us warm).
  - Output [64,1376] fp16 per core; host upcasts, rescales, concatenates.
"""

import numpy as np

OUT, IN, GS = 11008, 4096, 128
NG = IN // GS          # 32 groups
B = 64                 # batch rows
NCORES = 8
OPC = OUT // NCORES    # 1376 out features per core
CHUNKS = [(0, 512), (512, 512), (1024, OPC - 1024)]  # psum bank chunks
# weight DMA chunk sizes in K-groups: small first chunks so the cast+matmul
# pipeline fills fast, large later ones to amortize issue cost
CHUNK_GROUPS = [1, 1, 2, 2, 2, 4, 4, 4, 4, 4, 4]
assert sum(CHUNK_GROUPS) == NG
# int8->fp16 cast split. GpSimd compute is excluded: it shares an SBUF port
# with DVE under an exclusive lock, so concurrent GpSimd+DVE casts drop both
# to 1/4 rate (measured 5.4us vs 0.88us per [128,1376] tile), and each
# GpSimd op trails a ~3.5us DRAIN. Measured per-group cast: DVE 877ns
# (2x mode), ACT 1433ns. A few groups go out via SWDGE cast-during-DMA
# (int8 DRAM -> fp16 SBUF on the gpsimd queue) which costs no DVE/ACT
# cycles, only spare SDMA bandwidth. First groups go to DVE so the PE can
# start consuming immediately; the rest greedy least-loaded so completion
# order ~= group order (PE consumes in order).
_COST = {"vector": 877.0, "scalar": 1433.0}
SWDGE_GROUPS = (11, 15, 19, 23, 27, 31)


def _cast_assignment():
    load = {"vector": 0.0, "scalar": 0.0}
    assign = []
    for g in range(NG):
        if g in SWDGE_GROUPS:
            assign.append("swdge")
        elif g < 3:
            assign.append("vector")
            load["vector"] += _COST["vector"]
        else:
            eng = min(load, key=lambda e: load[e] + _COST[e])
            load[eng] += _COST[eng]
            assign.append(eng)
    return assign

_NC_CACHE = None


def _build():
    global _NC_CACHE
    if _NC_CACHE is not None:
        return _NC_CACHE

    import concourse.bacc as bacc
    import concourse.tile as tile
    import concourse.bass as bass
    import concourse.mybir as mybir

    f16 = mybir.dt.float16
    f32 = mybir.dt.float32
    i8 = mybir.dt.int8

    nc = bacc.Bacc(
        "TRN2", target_bir_lowering=False, debug=False, enable_asserts=False
    )
    # weight: partition-major int8, col g*OPC+o = w'[o, k=g*128+p] for part. p
    wt = nc.dram_tensor("wt", [128, NG * OPC], i8, kind="ExternalInput").ap()
    xg = nc.dram_tensor("xg", [128, NG * B], f16, kind="ExternalInput").ap()
    out = nc.dram_tensor("out", [B, OPC], f16, kind="ExternalOutput").ap()

    with tile.TileContext(nc) as tc:
        with (
            tc.tile_pool(name="xp", bufs=1) as xpool,
            tc.tile_pool(name="w8", bufs=len(CHUNK_GROUPS)) as w8pool,
            tc.tile_pool(name="wf", bufs=NG) as wfpool,
            tc.tile_pool(name="cp", bufs=1) as cpool,
            tc.tile_pool(name="op", bufs=1) as opool,
            tc.tile_pool(name="pp", bufs=1, space=bass.MemorySpace.PSUM) as pspool,
        ):
            # x on the scalar HWDGE ring so it doesn't delay weight chunks
            xt = xpool.tile([128, NG * B], f16)
            nc.scalar.dma_start(xt[:], xg[:])
            wsrc = cpool.tile([128, 256], f16, tag="wsrc")
            nc.gpsimd.memset(wsrc[:], 0.0)

            ps = [
                pspool.tile([B, n], f32, tag=f"ps{i}", name=f"ps{i}")
                for i, (_, n) in enumerate(CHUNKS)
            ]
            # HAM warm-up: back-to-back full-array matmuls while the first
            # weight chunks stream + cast, so the PE activity monitor
            # unthrottles 1.2->2.4GHz before real work (needs ~3.4us of
            # sustained PE busy) and the real matmuls follow seamlessly.
            warm_ps = pspool.tile([128, 256], f32, tag="warm", name="warm_ps")
            NWARM = 20
            for k in range(NWARM):
                nc.tensor.matmul(
                    warm_ps[:, :], wsrc[:, :128], wsrc[:, :],
                    start=(k == 0), stop=(k == NWARM - 1),
                )

            # weight DMA chunks (sync HWDGE ring), int8 staging tiles.
            # grp_loc[g] -> (chunk tile idx, group offset within chunk)
            w8 = []
            grp_loc = []
            g0 = 0
            for j, gpc in enumerate(CHUNK_GROUPS):
                t = w8pool.tile([128, gpc * OPC], i8)
                nc.sync.dma_start(t[:], wt[:, g0 * OPC : (g0 + gpc) * OPC])
                w8.append(t)
                for gp in range(gpc):
                    grp_loc.append((j, gp))
                g0 += gpc

            # per-group cast + matmul accumulation
            engines = {
                "vector": lambda o, i_: nc.vector.tensor_copy(o, i_),
                "scalar": lambda o, i_: nc.scalar.copy(o, i_),
            }
            assign = _cast_assignment()
            wf = []
            for g in range(NG):
                t = wfpool.tile([128, OPC], f16)
                if assign[g] == "swdge":
                    # cast-during-DMA straight from DRAM (SWDGE only)
                    nc.gpsimd.dma_start(t[:], wt[:, g * OPC : (g + 1) * OPC])
                else:
                    j, gp = grp_loc[g]
                    engines[assign[g]](t[:], w8[j][:, gp * OPC : (gp + 1) * OPC])
                wf.append(t)

            tail_order = [2, 0, 1]
            for g in range(NG):
                order = tail_order if g == NG - 1 else range(len(CHUNKS))
                for i in order:
                    o0, n = CHUNKS[i]
                    nc.tensor.matmul(
                        ps[i][:, :],
                        xt[:, g * B : (g + 1) * B],
                        wf[g][:, o0 : o0 + n],
                        start=(g == 0),
                        stop=(g == NG - 1),
                    )

            osb = opool.tile([B, OPC], f16)
            # copies split across vector+scalar; each chunk's store DMA issues
            # as soon as its copy lands so the tail overlaps.
            out_eng = [nc.sync, nc.scalar, nc.sync]
            for i, (o0, n) in enumerate(CHUNKS):
                if i == 1:
                    nc.scalar.copy(osb[:, o0 : o0 + n], ps[i][:, :])
                else:
                    nc.vector.tensor_copy(osb[:, o0 : o0 + n], ps[i][:, :])
                out_eng[i].dma_start(out[:, o0 : o0 + n], osb[:, o0 : o0 + n])

    nc.compile()
    _NC_CACHE = nc
    return nc


def _prep_inputs(x, weight_q, scale, bias, weight_residual):
    """Host-side requantize + shard + layout.

    Returns (in_maps, posts): per-core input dicts and per-core (S, add)
    fp32 arrays for the host-side affine out*S + add.
    """
    x = np.asarray(x, dtype=np.float32)
    weight_q = np.asarray(weight_q)
    scale = np.asarray(scale)
    bias = np.asarray(bias)
    weight_residual = np.asarray(weight_residual)
    # x [64, 4096] f32 -> [128 partitions(i within group), 32 groups, 64 batch] f16
    xgh = np.ascontiguousarray(
        x.reshape(B, NG, GS).transpose(2, 1, 0).astype(np.float16)
    ).reshape(128, NG * B)

    in_maps = []
    posts = []
    for c in range(NCORES):
        rows = slice(c * OPC, (c + 1) * OPC)
        wq_c = weight_q[rows]                       # [1376, 4096] int8
        sc_c = scale[rows].astype(np.float32)       # [1376, 32]
        wd = (
            wq_c.reshape(OPC, NG, GS).astype(np.float32)
            * sc_c[:, :, None]
        ).reshape(OPC, IN)
        S = sc_c.max(axis=1)                        # [1376] > 0
        w8 = np.rint(wd / S[:, None]).astype(np.int8)   # |.| <= 127 by constr.
        # [4096, 1376] -> partition-major [128, 32*1376]
        wt_c = np.ascontiguousarray(
            w8.T.reshape(NG, 128, OPC).transpose(1, 0, 2).reshape(128, NG * OPC)
        )
        add_c = (
            bias[rows].astype(np.float32)
            + weight_residual[rows].astype(np.float32)
        )
        in_maps.append({"wt": wt_c, "xg": xgh})
        posts.append((S, add_c))
    return in_maps, posts


def kernel(x, weight_q, scale, bias, weight_residual):
    from concourse.bass_utils import run_bass_kernel_spmd

    nc = _build()
    in_maps, posts = _prep_inputs(x, weight_q, scale, bias, weight_residual)
    for _attempt in range(3):
        res = run_bass_kernel_spmd(nc, in_maps, core_ids=list(range(NCORES)))
        out = np.concatenate(
            [
                res.results[c]["out"].astype(np.float32) * posts[c][0][None, :]
                + posts[c][1][None, :]
                for c in range(NCORES)
            ],
            axis=1,
        )
        # guard against a rare transient on a freshly-loaded NEFF
        if np.isfinite(out).all():
            return out
    return out


# revision 14
# speedup vs baseline: 1.0498x; 1.0498x over previous
"""Trainium2 Bass kernel for nn_DynamicQuantizedLinear.

Computes out = x @ dequant(W).T + bias + residual where
  x:[64,4096] f32, W_q:[11008,4096] int8, scale:[11008,32] f16 (group size 128),
  bias/residual:[11008] f16.

Strategy (column-parallel over out_features, 8 cores):
  - Host: requantize the dequantized weights to a SINGLE int8 scale per
    output row: S[o] = max_g scale[o,g], w' = rint(dequant(W)/S) in [-127,127].
    This removes all per-group scaling from the device (rel err ~6e-3 vs
    2e-2 tolerance). Host applies out*S + bias + residual afterwards (free).
  - Device: weights stream as INT8 (5.6MB/core, half the fp16 bytes, ~14us
    at ~410GB/s) in 16 column chunks; DVE/ACT/GPSIMD round-robin cast each
    [128,1376] group slab int8->fp16 (~14.5us/engine, overlapped); PE
    accumulates all 32 K-groups into 3 PSUM banks (N=512/512/352) with a
    [128,64] fp16 x-tile stationary per group (~18.4us warm).
  - Output [64,1376] fp16 per core; host upcasts, rescales, concatenates.
"""

import numpy as np

OUT, IN, GS = 11008, 4096, 128
NG = IN // GS          # 32 groups
B = 64                 # batch rows
NCORES = 8
OPC = OUT // NCORES    # 1376 out features per core
CHUNKS = [(0, 512), (512, 512), (1024, OPC - 1024)]  # psum bank chunks
# weight DMA chunk sizes in K-groups: small first chunks so the cast+matmul
# pipeline fills fast, large later ones to amortize issue cost
CHUNK_GROUPS = [1, 1, 2, 2, 2, 4, 4, 4, 4, 4, 4]
assert sum(CHUNK_GROUPS) == NG
# int8->fp16 cast split. GpSimd compute is excluded: it shares an SBUF port
# with DVE under an exclusive lock, so concurrent GpSimd+DVE casts drop both
# to 1/4 rate (measured 5.4us vs 0.88us per [128,1376] tile), and each
# GpSimd op trails a ~3.5us DRAIN. Measured per-group cast: DVE 877ns
# (2x mode), ACT 1433ns. A few groups go out via SWDGE cast-during-DMA
# (int8 DRAM -> fp16 SBUF on the gpsimd queue) which costs no DVE/ACT
# cycles, only spare SDMA bandwidth. First groups go to DVE so the PE can
# start consuming immediately; the rest greedy least-loaded so completion
# order ~= group order (PE consumes in order).
_COST = {"vector": 877.0, "scalar": 1433.0}
# SWDGE cast-during-DMA offload was tried and REGRESSED (+5.7us): its SDMA
# transfers compete with the critical HWDGE weight stream and delay chunks.
SWDGE_GROUPS = ()


def _cast_assignment():
    load = {"vector": 0.0, "scalar": 0.0}
    assign = []
    for g in range(NG):
        if g in SWDGE_GROUPS:
            assign.append("swdge")
        elif g < 3:
            assign.append("vector")
            load["vector"] += _COST["vector"]
        else:
            eng = min(load, key=lambda e: load[e] + _COST[e])
            load[eng] += _COST[eng]
            assign.append(eng)
    return assign

_NC_CACHE = None


def _build():
    global _NC_CACHE
    if _NC_CACHE is not None:
        return _NC_CACHE

    import concourse.bacc as bacc
    import concourse.tile as tile
    import concourse.bass as bass
    import concourse.mybir as mybir

    f16 = mybir.dt.float16
    f32 = mybir.dt.float32
    i8 = mybir.dt.int8

    nc = bacc.Bacc(
        "TRN2", target_bir_lowering=False, debug=False, enable_asserts=False
    )
    # weight: partition-major int8, col g*OPC+o = w'[o, k=g*128+p] for part. p
    wt = nc.dram_tensor("wt", [128, NG * OPC], i8, kind="ExternalInput").ap()
    xg = nc.dram_tensor("xg", [128, NG * B], f16, kind="ExternalInput").ap()
    out = nc.dram_tensor("out", [B, OPC], f16, kind="ExternalOutput").ap()

    with tile.TileContext(nc) as tc:
        with (
            tc.tile_pool(name="xp", bufs=1) as xpool,
            tc.tile_pool(name="w8", bufs=len(CHUNK_GROUPS)) as w8pool,
            tc.tile_pool(name="wf", bufs=NG) as wfpool,
            tc.tile_pool(name="cp", bufs=1) as cpool,
            tc.tile_pool(name="op", bufs=1) as opool,
            tc.tile_pool(name="pp", bufs=1, space=bass.MemorySpace.PSUM) as pspool,
        ):
            # x on the scalar HWDGE ring so it doesn't delay weight chunks;
            # split in 4 so the first groups' columns land (incl. the ~2us
            # DMA completion latency) before the PE needs them
            xt = xpool.tile([128, NG * B], f16)
            XSPLIT = 4
            xn = NG * B // XSPLIT
            for j in range(XSPLIT):
                nc.scalar.dma_start(
                    xt[:, j * xn : (j + 1) * xn], xg[:, j * xn : (j + 1) * xn]
                )
            wsrc = cpool.tile([128, 256], f16, tag="wsrc")
            nc.gpsimd.memset(wsrc[:], 0.0)

            ps = [
                pspool.tile([B, n], f32, tag=f"ps{i}", name=f"ps{i}")
                for i, (_, n) in enumerate(CHUNKS)
            ]
            # HAM warm-up: back-to-back full-array matmuls while the first
            # weight chunks stream + cast, so the PE activity monitor
            # unthrottles 1.2->2.4GHz before real work (needs ~3.4us of
            # sustained PE busy) and the real matmuls follow seamlessly.
            warm_ps = pspool.tile([128, 256], f32, tag="warm", name="warm_ps")
            NWARM = 18
            for k in range(NWARM):
                nc.tensor.matmul(
                    warm_ps[:, :], wsrc[:, :128], wsrc[:, :],
                    start=(k == 0), stop=(k == NWARM - 1),
                )

            # weight DMA chunks (sync HWDGE ring), int8 staging tiles.
            # grp_loc[g] -> (chunk tile idx, group offset within chunk)
            w8 = []
            grp_loc = []
            g0 = 0
            for j, gpc in enumerate(CHUNK_GROUPS):
                t = w8pool.tile([128, gpc * OPC], i8)
                nc.sync.dma_start(t[:], wt[:, g0 * OPC : (g0 + gpc) * OPC])
                w8.append(t)
                for gp in range(gpc):
                    grp_loc.append((j, gp))
                g0 += gpc

            # per-group cast + matmul accumulation
            engines = {
                "vector": lambda o, i_: nc.vector.tensor_copy(o, i_),
                "scalar": lambda o, i_: nc.scalar.copy(o, i_),
            }
            assign = _cast_assignment()
            wf = []
            for g in range(NG):
                t = wfpool.tile([128, OPC], f16)
                if assign[g] == "swdge":
                    # cast-during-DMA straight from DRAM (SWDGE only)
                    nc.gpsimd.dma_start(t[:], wt[:, g * OPC : (g + 1) * OPC])
                else:
                    j, gp = grp_loc[g]
                    engines[assign[g]](t[:], w8[j][:, gp * OPC : (gp + 1) * OPC])
                wf.append(t)

            tail_order = [2, 0, 1]
            for g in range(NG):
                order = tail_order if g == NG - 1 else range(len(CHUNKS))
                for i in order:
                    o0, n = CHUNKS[i]
                    nc.tensor.matmul(
                        ps[i][:, :],
                        xt[:, g * B : (g + 1) * B],
                        wf[g][:, o0 : o0 + n],
                        start=(g == 0),
                        stop=(g == NG - 1),
                    )

            osb = opool.tile([B, OPC], f16)
            # copies split across vector+scalar; each chunk's store DMA issues
            # as soon as its copy lands so the tail overlaps.
            out_eng = [nc.sync, nc.scalar, nc.sync]
            for i, (o0, n) in enumerate(CHUNKS):
                if i == 1:
                    nc.scalar.copy(osb[:, o0 : o0 + n], ps[i][:, :])
                else:
                    nc.vector.tensor_copy(osb[:, o0 : o0 + n], ps[i][:, :])
                out_eng[i].dma_start(out[:, o0 : o0 + n], osb[:, o0 : o0 + n])

    nc.compile()
    _NC_CACHE = nc
    return nc


def _prep_inputs(x, weight_q, scale, bias, weight_residual):
    """Host-side requantize + shard + layout.

    Returns (in_maps, posts): per-core input dicts and per-core (S, add)
    fp32 arrays for the host-side affine out*S + add.
    """
    x = np.asarray(x, dtype=np.float32)
    weight_q = np.asarray(weight_q)
    scale = np.asarray(scale)
    bias = np.asarray(bias)
    weight_residual = np.asarray(weight_residual)
    # x [64, 4096] f32 -> [128 partitions(i within group), 32 groups, 64 batch] f16
    xgh = np.ascontiguousarray(
        x.reshape(B, NG, GS).transpose(2, 1, 0).astype(np.float16)
    ).reshape(128, NG * B)

    in_maps = []
    posts = []
    for c in range(NCORES):
        rows = slice(c * OPC, (c + 1) * OPC)
        wq_c = weight_q[rows]                       # [1376, 4096] int8
        sc_c = scale[rows].astype(np.float32)       # [1376, 32]
        wd = (
            wq_c.reshape(OPC, NG, GS).astype(np.float32)
            * sc_c[:, :, None]
        ).reshape(OPC, IN)
        S = sc_c.max(axis=1)                        # [1376] > 0
        w8 = np.rint(wd / S[:, None]).astype(np.int8)   # |.| <= 127 by constr.
        # [4096, 1376] -> partition-major [128, 32*1376]
        wt_c = np.ascontiguousarray(
            w8.T.reshape(NG, 128, OPC).transpose(1, 0, 2).reshape(128, NG * OPC)
        )
        add_c = (
            bias[rows].astype(np.float32)
            + weight_residual[rows].astype(np.float32)
        )
        in_maps.append({"wt": wt_c, "xg": xgh})
        posts.append((S, add_c))
    return in_maps, posts


def kernel(x, weight_q, scale, bias, weight_residual):
    from concourse.bass_utils import run_bass_kernel_spmd

    nc = _build()
    in_maps, posts = _prep_inputs(x, weight_q, scale, bias, weight_residual)
    for _attempt in range(3):
        res = run_bass_kernel_spmd(nc, in_maps, core_ids=list(range(NCORES)))
        out = np.concatenate(
            [
                res.results[c]["out"].astype(np.float32) * posts[c][0][None, :]
                + posts[c][1][None, :]
                for c in range(NCORES)
            ],
            axis=1,
        )
        # guard against a rare transient on a freshly-loaded NEFF
        if np.isfinite(out).all():
            return out
    return out


# revision 16
# speedup vs baseline: 1.1448x; 1.0905x over previous
"""Trainium2 Bass kernel for nn_DynamicQuantizedLinear.

Computes out = x @ dequant(W).T + bias + residual where
  x:[64,4096] f32, W_q:[11008,4096] int8, scale:[11008,32] f16 (group size 128),
  bias/residual:[11008] f16.

Strategy (column-parallel over out_features, 8 cores):
  - Host: requantize the dequantized weights to a SINGLE int8 scale per
    output row: S[o] = max_g scale[o,g], w' = rint(dequant(W)/S) in [-127,127].
    This removes all per-group scaling from the device (rel err ~6e-3 vs
    2e-2 tolerance). Host applies out*S + bias + residual afterwards (free).
  - Device: weights stream as INT8 (5.6MB/core, half the fp16 bytes, ~14us
    at ~410GB/s) in 16 column chunks; DVE/ACT/GPSIMD round-robin cast each
    [128,1376] group slab int8->fp16 (~14.5us/engine, overlapped); PE
    accumulates all 32 K-groups into 3 PSUM banks (N=512/512/352) with a
    [128,64] fp16 x-tile stationary per group (~18.4us warm).
  - Output [64,1376] fp16 per core; host upcasts, rescales, concatenates.
"""

import numpy as np

OUT, IN, GS = 11008, 4096, 128
NG = IN // GS          # 32 groups
B = 64                 # batch rows
NCORES = 8
OPC = OUT // NCORES    # 1376 out features per core
CHUNKS = [(0, 512), (512, 512), (1024, OPC - 1024)]  # psum bank chunks
# weight DMA chunk sizes in K-groups: small first chunks so the cast+matmul
# pipeline fills fast, large later ones to amortize issue cost
CHUNK_GROUPS = [1, 1, 2, 2, 2, 4, 4, 4, 4, 4, 4]
assert sum(CHUNK_GROUPS) == NG
# int8->fp16 cast split. GpSimd compute is excluded: it shares an SBUF port
# with DVE under an exclusive lock, so concurrent GpSimd+DVE casts drop both
# to 1/4 rate (measured 5.4us vs 0.88us per [128,1376] tile), and each
# GpSimd op trails a ~3.5us DRAIN. Measured per-group cast: DVE 877ns
# (2x mode), ACT 1433ns. A few groups go out via SWDGE cast-during-DMA
# (int8 DRAM -> fp16 SBUF on the gpsimd queue) which costs no DVE/ACT
# cycles, only spare SDMA bandwidth. First groups go to DVE so the PE can
# start consuming immediately; the rest greedy least-loaded so completion
# order ~= group order (PE consumes in order).
_COST = {"vector": 877.0, "scalar": 1433.0}
# SWDGE cast-during-DMA offload was tried and REGRESSED (+5.7us): its SDMA
# transfers compete with the critical HWDGE weight stream and delay chunks.
SWDGE_GROUPS = ()


def _cast_assignment():
    # rate-proportional interleave, 20 DVE : 12 ACT, phase-shifted so g0 is
    # DVE and neither engine gets back-to-back groups (which would stall the
    # PE, which consumes a group every ~573ns while DVE/ACT produce one
    # every 877/1433ns)
    assign = []
    for g in range(NG):
        if g in SWDGE_GROUPS:
            assign.append("swdge")
        elif ((g + 1) * 5 + 4) // 8 > (g * 5 + 4) // 8:
            assign.append("vector")
        else:
            assign.append("scalar")
    return assign

_NC_CACHE = None


def _build():
    global _NC_CACHE
    if _NC_CACHE is not None:
        return _NC_CACHE

    import concourse.bacc as bacc
    import concourse.tile as tile
    import concourse.bass as bass
    import concourse.mybir as mybir

    f16 = mybir.dt.float16
    f32 = mybir.dt.float32
    i8 = mybir.dt.int8

    nc = bacc.Bacc(
        "TRN2", target_bir_lowering=False, debug=False, enable_asserts=False
    )
    # weight: partition-major int8, col g*OPC+o = w'[o, k=g*128+p] for part. p
    wt = nc.dram_tensor("wt", [128, NG * OPC], i8, kind="ExternalInput").ap()
    xg = nc.dram_tensor("xg", [128, NG * B], f16, kind="ExternalInput").ap()
    out = nc.dram_tensor("out", [B, OPC], f16, kind="ExternalOutput").ap()

    with tile.TileContext(nc) as tc:
        with (
            tc.tile_pool(name="xp", bufs=1) as xpool,
            tc.tile_pool(name="w8", bufs=len(CHUNK_GROUPS)) as w8pool,
            tc.tile_pool(name="wf", bufs=NG) as wfpool,
            tc.tile_pool(name="cp", bufs=1) as cpool,
            tc.tile_pool(name="op", bufs=1) as opool,
            tc.tile_pool(name="pp", bufs=1, space=bass.MemorySpace.PSUM) as pspool,
        ):
            # x on the scalar HWDGE ring so it doesn't delay weight chunks;
            # split in 4 so the first groups' columns land (incl. the ~2us
            # DMA completion latency) before the PE needs them
            xt = xpool.tile([128, NG * B], f16)
            XSPLIT = 2
            xn = NG * B // XSPLIT
            for j in range(XSPLIT):
                nc.scalar.dma_start(
                    xt[:, j * xn : (j + 1) * xn], xg[:, j * xn : (j + 1) * xn]
                )
            wsrc = cpool.tile([128, 256], f16, tag="wsrc")
            nc.gpsimd.memset(wsrc[:], 0.0)

            ps = [
                pspool.tile([B, n], f32, tag=f"ps{i}", name=f"ps{i}")
                for i, (_, n) in enumerate(CHUNKS)
            ]
            # HAM warm-up: back-to-back full-array matmuls while the first
            # weight chunks stream + cast, so the PE activity monitor
            # unthrottles 1.2->2.4GHz before real work (needs ~3.4us of
            # sustained PE busy) and the real matmuls follow seamlessly.
            warm_ps = pspool.tile([128, 256], f32, tag="warm", name="warm_ps")
            NWARM = 18
            for k in range(NWARM):
                nc.tensor.matmul(
                    warm_ps[:, :], wsrc[:, :128], wsrc[:, :],
                    start=(k == 0), stop=(k == NWARM - 1),
                )

            # weight DMA chunks (sync HWDGE ring), int8 staging tiles.
            # grp_loc[g] -> (chunk tile idx, group offset within chunk)
            w8 = []
            grp_loc = []
            g0 = 0
            for j, gpc in enumerate(CHUNK_GROUPS):
                t = w8pool.tile([128, gpc * OPC], i8)
                nc.sync.dma_start(t[:], wt[:, g0 * OPC : (g0 + gpc) * OPC])
                w8.append(t)
                for gp in range(gpc):
                    grp_loc.append((j, gp))
                g0 += gpc

            # per-group cast + matmul accumulation
            engines = {
                "vector": lambda o, i_: nc.vector.tensor_copy(o, i_),
                "scalar": lambda o, i_: nc.scalar.copy(o, i_),
            }
            assign = _cast_assignment()
            wf = []
            for g in range(NG):
                t = wfpool.tile([128, OPC], f16)
                if assign[g] == "swdge":
                    # cast-during-DMA straight from DRAM (SWDGE only)
                    nc.gpsimd.dma_start(t[:], wt[:, g * OPC : (g + 1) * OPC])
                else:
                    j, gp = grp_loc[g]
                    engines[assign[g]](t[:], w8[j][:, gp * OPC : (gp + 1) * OPC])
                wf.append(t)

            tail_order = [2, 0, 1]
            for g in range(NG):
                order = tail_order if g == NG - 1 else range(len(CHUNKS))
                for i in order:
                    o0, n = CHUNKS[i]
                    nc.tensor.matmul(
                        ps[i][:, :],
                        xt[:, g * B : (g + 1) * B],
                        wf[g][:, o0 : o0 + n],
                        start=(g == 0),
                        stop=(g == NG - 1),
                    )

            osb = opool.tile([B, OPC], f16)
            # copies split across vector+scalar; each chunk's store DMA issues
            # as soon as its copy lands so the tail overlaps.
            out_eng = [nc.sync, nc.scalar, nc.sync]
            for i, (o0, n) in enumerate(CHUNKS):
                if i == 1:
                    nc.scalar.copy(osb[:, o0 : o0 + n], ps[i][:, :])
                else:
                    nc.vector.tensor_copy(osb[:, o0 : o0 + n], ps[i][:, :])
                out_eng[i].dma_start(out[:, o0 : o0 + n], osb[:, o0 : o0 + n])

    nc.compile()
    _NC_CACHE = nc
    return nc


def _prep_inputs(x, weight_q, scale, bias, weight_residual):
    """Host-side requantize + shard + layout.

    Returns (in_maps, posts): per-core input dicts and per-core (S, add)
    fp32 arrays for the host-side affine out*S + add.
    """
    x = np.asarray(x, dtype=np.float32)
    weight_q = np.asarray(weight_q)
    scale = np.asarray(scale)
    bias = np.asarray(bias)
    weight_residual = np.asarray(weight_residual)
    # x [64, 4096] f32 -> [128 partitions(i within group), 32 groups, 64 batch] f16
    xgh = np.ascontiguousarray(
        x.reshape(B, NG, GS).transpose(2, 1, 0).astype(np.float16)
    ).reshape(128, NG * B)

    in_maps = []
    posts = []
    for c in range(NCORES):
        rows = slice(c * OPC, (c + 1) * OPC)
        wq_c = weight_q[rows]                       # [1376, 4096] int8
        sc_c = scale[rows].astype(np.float32)       # [1376, 32]
        wd = (
            wq_c.reshape(OPC, NG, GS).astype(np.float32)
            * sc_c[:, :, None]
        ).reshape(OPC, IN)
        S = sc_c.max(axis=1)                        # [1376] > 0
        w8 = np.rint(wd / S[:, None]).astype(np.int8)   # |.| <= 127 by constr.
        # [4096, 1376] -> partition-major [128, 32*1376]
        wt_c = np.ascontiguousarray(
            w8.T.reshape(NG, 128, OPC).transpose(1, 0, 2).reshape(128, NG * OPC)
        )
        add_c = (
            bias[rows].astype(np.float32)
            + weight_residual[rows].astype(np.float32)
        )
        in_maps.append({"wt": wt_c, "xg": xgh})
        posts.append((S, add_c))
    return in_maps, posts


def kernel(x, weight_q, scale, bias, weight_residual):
    from concourse.bass_utils import run_bass_kernel_spmd

    nc = _build()
    in_maps, posts = _prep_inputs(x, weight_q, scale, bias, weight_residual)
    for _attempt in range(3):
        res = run_bass_kernel_spmd(nc, in_maps, core_ids=list(range(NCORES)))
        out = np.concatenate(
            [
                res.results[c]["out"].astype(np.float32) * posts[c][0][None, :]
                + posts[c][1][None, :]
                for c in range(NCORES)
            ],
            axis=1,
        )
        # guard against a rare transient on a freshly-loaded NEFF
        if np.isfinite(out).all():
            return out
    return out
